# revision 24
# baseline (speedup 1.0000x reference)
"""Bi-directional cross-attention kernel for Trainium2 (8 NeuronCores).

Sharding: data-parallel over batch B=8 -> one batch element per core (SPMD,
no collectives). Each core computes the full bidirectional cross-attention
for its batch element.

Per-core layout strategy (C=256 channels, S=1024 tokens, 8 heads x 64 dim):

  - K1col/K2col: [512, 1024] (head-major rows on partitions, tokens free).
  - V1aug/V2aug: row layout [1024 tokens, 8*(64+1)] with a ones-column per
    head; the ones-column makes the attention matmuls emit the softmax
    denominator as an extra output COLUMN for free.
  - scores per (pair, q-chunk) with the two heads of a pair on PE array row
    groups 0-63 / 64-127 and their matmuls ALTERNATED so the two K=64
    streams overlap in the PE quadrants (measured on HW: 211 vs 501
    ns/matmul at N=512). exp(0.125*s) fused into the PSUM->SBUF copy on
    ScalarE (scores are tiny, no max-shift needed).
  - E^T via xbar DMA transposes on the SP HWDGE ring (SP ring ONLY: the ACT
    HWDGE ring corrupts transposed data on this runtime -- verified).
  - attention outputs computed in ROW layout (tokens on partitions):
      out2row[k, d] = sum_q E[q,k] V1aug[q,d]   (lhsT = E chunk,  N=65)
      out1row[q, d] = sum_k Et[k,q] V2aug[k,d]  (lhsT = Et chunk, N=65)
    The denominator lands in column 64 of each head block. (The
    column-layout formulation with N=512 matmuls plus matmul-broadcast
    normalize measured ~28us SLOWER end-to-end; interleaving these chains
    with the score matmuls as open PSUM accumulation groups produced
    wrong results on HW -- accumulation groups must stay contiguous.)
  - normalize: one DVE reciprocal on the [128, 2] denominator columns, then
    a single broadcast (stride-0) tensor_tensor multiply per site on DVE,
    writing normalized bf16 rows. Pool cannot touch PSUM on this target.
  - row->column conversion of the normalized outputs via ONE xbar DMA
    transpose per (pair, output) -- tiny (2MB total) next to the E^T stream.
  - output projection consumes the column tiles per head PAIR (K=128
    contraction); biases (incl. folded V-bias) applied per-partition during
    the final PSUM->SBUF copy on VectorE.
  - iters>1 wraps the body in a tc.For_i hardware loop (body emitted once,
    replayed on device) -- used by test.py to measure marginal per-iteration
    HW time without NEFF-size-dependent host overhead.
"""

import os
import sys

for _p in ("/opt/trn_rl_repo", os.path.expanduser("~/.axon_site/_ro/trn_rl_repo")):
    if os.path.isdir(_p) and _p not in sys.path:
        sys.path.insert(0, _p)

import numpy as np
import ml_dtypes

import concourse.bass as bass
import concourse.tile as tile
import concourse.mybir as mybir
from concourse import bacc

BF16 = mybir.dt.bfloat16
F32 = mybir.dt.float32
AF = mybir.ActivationFunctionType
ALU = mybir.AluOpType

B = 8
C = 256          # channels per image
S = 1024         # tokens per image (32*32)
NH = 8           # heads
HD = 64          # head dim
J = NH * HD      # 512
P = 128
NCC = C // P     # 2 channel chunks
NQ = S // P      # 8 token chunks
NKB = S // 512   # 2 psum banks across tokens
HB = HD + 1      # head block width in V-aug (64 d + ones col)
NP = NH // 2     # head pairs


def _emit(nc: bass.Bass, iters: int = 1, unroll: int = 1) -> None:
    x1 = nc.declare_dram_parameter("x1", [C, S], BF16, isOutput=False)
    x2 = nc.declare_dram_parameter("x2", [C, S], BF16, isOutput=False)
    wk1 = nc.declare_dram_parameter("wk1", [C, J], BF16, isOutput=False)
    wk2 = nc.declare_dram_parameter("wk2", [C, J], BF16, isOutput=False)
    wv1 = nc.declare_dram_parameter("wv1", [C, J], BF16, isOutput=False)
    wv2 = nc.declare_dram_parameter("wv2", [C, J], BF16, isOutput=False)
    wo1 = nc.declare_dram_parameter("wo1", [J, C], BF16, isOutput=False)
    wo2 = nc.declare_dram_parameter("wo2", [J, C], BF16, isOutput=False)
    bk1 = nc.declare_dram_parameter("bk1", [P, J // P], F32, isOutput=False)
    bk2 = nc.declare_dram_parameter("bk2", [P, J // P], F32, isOutput=False)
    bo1 = nc.declare_dram_parameter("bo1", [P, NCC], F32, isOutput=False)
    bo2 = nc.declare_dram_parameter("bo2", [P, NCC], F32, isOutput=False)
    y1 = nc.declare_dram_parameter("y1", [C, S], F32, isOutput=True)
    y2 = nc.declare_dram_parameter("y2", [C, S], F32, isOutput=True)

    with tile.TileContext(nc) as tc:
        with (
            tc.tile_pool(name="const", bufs=1) as cp,
            tc.tile_pool(name="work", bufs=2) as wp,
            tc.tile_pool(name="norm", bufs=4) as np_,
            # psA: 2 x [128,1024] f32 score/projection buffers (4 banks);
            # psB: 2 x [128,512] attention-out chain buffers (2 banks)
            tc.tile_pool(name="psA", bufs=2, space="PSUM") as psA,
            tc.tile_pool(name="psB", bufs=2, space="PSUM") as psB,
        ):
            def _body_loop():
                if iters == 1:
                    yield 0
                else:
                    # hardware loop: one body copy in the NEFF, replayed
                    # iters times back-to-back on device. unroll>1 emits the
                    # body several times per loop trip (fewer back-edge
                    # all-engine barriers).
                    assert iters % unroll == 0
                    with tc.For_i(0, iters // unroll, 1):
                        for _u in range(unroll):
                            yield _u

            def load(dram, shape, dtype, tag):
                # SWDGE (Pool-issued) ring: keeps the SP HWDGE ring free
                # for the xbar transposes
                t = cp.tile(shape, dtype, tag=tag, name=tag)
                nc.gpsimd.dma_start(out=t[:], in_=dram[:])
                return t

            # weights and biases are loop-invariant: loaded once, resident in
            # SBUF across iterations (weight-stationary steady state)
            wk1_sb = [load(wk1[cc * P:(cc + 1) * P, :], [P, J], BF16, f"wk1_{cc}")
                      for cc in range(NCC)]
            wk2_sb = [load(wk2[cc * P:(cc + 1) * P, :], [P, J], BF16, f"wk2_{cc}")
                      for cc in range(NCC)]
            wv1_sb = [load(wv1[cc * P:(cc + 1) * P, :], [P, J], BF16, f"wv1_{cc}")
                      for cc in range(NCC)]
            wv2_sb = [load(wv2[cc * P:(cc + 1) * P, :], [P, J], BF16, f"wv2_{cc}")
                      for cc in range(NCC)]
            # o-projection weights: one [128, C] tile per head PAIR so the
            # out-projection contracts K=128 per matmul.
            wo1_sb = [load(wo1[hp * P:(hp + 1) * P, :], [P, C], BF16, f"wo1_{hp}")
                      for hp in range(NP)]
            wo2_sb = [load(wo2[hp * P:(hp + 1) * P, :], [P, C], BF16, f"wo2_{hp}")
                      for hp in range(NP)]
            bk1_sb = load(bk1, [P, J // P], F32, "bk1")
            bk2_sb = load(bk2, [P, J // P], F32, "bk2")
            bo1_sb = load(bo1, [P, NCC], F32, "bo1")
            bo2_sb = load(bo2, [P, NCC], F32, "bo2")

            for _it in _body_loop():
                # ---- load activations -------------------------------------------
                def load2(dram, shape, dtype, tag):
                    # bufs=2 so the next loop copy's loads overlap this
                    # copy's compute tail
                    t = cp.tile(shape, dtype, tag=tag, name=tag, bufs=2)
                    nc.gpsimd.dma_start(out=t[:], in_=dram[:])
                    return t

                x1_sb = [load2(x1[cc * P:(cc + 1) * P, :], [P, S], BF16, f"x1_{cc}")
                         for cc in range(NCC)]
                x2_sb = [load2(x2[cc * P:(cc + 1) * P, :], [P, S], BF16, f"x2_{cc}")
                         for cc in range(NCC)]

                # ---- K projections: Kcol[j, s] = sum_c wk[c, j] * x[c, s] + bk ---
                def k_proj(x_sb, w_sb, b_sb, tag):
                    out = []
                    for m in range(J // P):
                        ps = psA.tile([P, S], F32, tag="pe", name="pe")
                        for nb in range(NKB):
                            for cc in range(NCC):
                                nc.tensor.matmul(
                                    ps[:, nb * 512:(nb + 1) * 512],
                                    lhsT=w_sb[cc][:, m * P:(m + 1) * P],
                                    rhs=x_sb[cc][:, nb * 512:(nb + 1) * 512],
                                    start=(cc == 0), stop=(cc == NCC - 1),
                                )
                        k_sb = cp.tile([P, S], BF16, tag=f"{tag}_{m}", name=f"{tag}_{m}")
                        nc.vector.tensor_scalar(k_sb[:], ps[:],
                                                b_sb[:, m:m + 1], None, ALU.add)
                        out.append(k_sb)
                    return out

                K1_sb = k_proj(x1_sb, wk1_sb, bk1_sb, "k1")
                K2_sb = k_proj(x2_sb, wk2_sb, bk2_sb, "k2")

                # ---- V projections into augmented row layout ---------------------
                # Vaug[qc] : [128 tokens, 8*(64+1)] ; per-head 64 values + ones col
                def v_proj(x_sb, w_sb, tag):
                    out = []
                    for qc in range(NQ):
                        # uses the psA "pe" ring (psB is reserved for the 8
                        # open attention accumulation chains)
                        ps = psA.tile([P, S], F32, tag="pe", name="pe")
                        for cc in range(NCC):
                            nc.tensor.matmul(
                                ps[:, 0:J],
                                lhsT=x_sb[cc][:, qc * P:(qc + 1) * P],
                                rhs=w_sb[cc][:],
                                start=(cc == 0), stop=(cc == NCC - 1),
                            )
                        va = cp.tile([P, NH * HB], BF16, tag=f"{tag}_{qc}", name=f"{tag}_{qc}")
                        va_v = va[:].rearrange("p (h c) -> p h c", c=HB)
                        ps_v = ps[:, 0:J].rearrange("p (h c) -> p h c", c=HD)
                        # staging copy on DVE (Pool cannot access PSUM)
                        nc.vector.tensor_copy(va_v[:, :, 0:HD], ps_v)
                        nc.gpsimd.memset(va_v[:, :, HD:HB], 1.0)
                        out.append(va)
                    return out

                V1a_sb = v_proj(x1_sb, wv1_sb, "v1a")
                V2a_sb = v_proj(x2_sb, wv2_sb, "v2a")

                # column-layout output tiles per head PAIR: [128 j, S] (head A
                # dims 0-63, head B dims 64-127) so out_proj contracts K=128.
                O1c = [cp.tile([P, S], BF16, tag=f"o1_{hp}", name=f"o1_{hp}")
                       for hp in range(NP)]
                O2c = [cp.tile([P, S], BF16, tag=f"o2_{hp}", name=f"o2_{hp}")
                       for hp in range(NP)]

                def normalize_row(po, orow, site, parity):
                    """po: [128 tokens, 130] psum: per pair-head hh, columns
                    hh*65..hh*65+63 hold the unnormalized output rows and
                    column hh*65+64 the softmax denominator. Writes the
                    normalized bf16 rows into orow[:, site, :]. All on DVE
                    (Pool cannot access PSUM): one reciprocal on the two
                    denominator columns, one broadcast multiply."""
                    po_v = po[:, 0:2 * HB].rearrange("p (h c) -> p h c", c=HB)
                    r = np_.tile([P, 2], F32, tag="r", name="r", bufs=8)
                    r_v = r[:].rearrange("p (h c) -> p h c", c=1)
                    nc.vector.reciprocal(out=r_v[:], in_=po_v[:, :, HD:HB])
                    out_v = orow[:, site, :].rearrange("p (h c) -> p h c", c=HD)
                    i0, i1 = bass.broadcast_tensor_aps(po_v[:, :, 0:HD], r_v)
                    nc.vector.tensor_tensor(out=out_v, in0=i0, in1=i1,
                                            op=ALU.mult)

                # ---- attention, head pairs, SOFTWARE-PIPELINED ------------------
                # Pair hp's scores/exp phase is ACT-bound (~2.1us/step on the
                # scalar engine vs ~0.9us of PE score work), while the
                # attention-out chains are PE-bound with ACT idle. So pair
                # hp-1's out-chains are emitted one SITE per q-chunk step of
                # pair hp: each site is a contiguous PSUM accumulation chain
                # (interleaving matmuls INSIDE a chain corrupts accumulation
                # on HW), but whole chains slot safely between score matmuls
                # and fill the PE idle window under pair hp's exps.

                def emit_site_out2(state, s):
                    """out2row site s (k-chunk s) for a completed pair."""
                    pair, e_sb, et_qc, or1, or2, hp = state
                    po = psB.tile([P, 512], F32, tag="po", name="po")
                    for hh, h in enumerate(pair):
                        for qc in range(NQ):
                            nc.tensor.matmul(
                                po[:, hh * HB:(hh + 1) * HB],
                                lhsT=e_sb[h][qc][:, s * P:(s + 1) * P],
                                rhs=V1a_sb[qc][:, h * HB:(h + 1) * HB],
                                start=(qc == 0), stop=(qc == NQ - 1),
                            )
                    normalize_row(po, or2, s, s % 2)

                def emit_site_out1(state, s):
                    """out1row site s (q-chunk s) for a completed pair."""
                    pair, e_sb, et_qc, or1, or2, hp = state
                    po = psB.tile([P, 512], F32, tag="po", name="po")
                    for hh, h in enumerate(pair):
                        for kc in range(NQ):
                            nc.tensor.matmul(
                                po[:, hh * HB:(hh + 1) * HB],
                                lhsT=et_qc[h][s][:, kc, :],
                                rhs=V2a_sb[kc][:, h * HB:(h + 1) * HB],
                                start=(kc == 0), stop=(kc == NQ - 1),
                            )
                    normalize_row(po, or1, s, s % 2)

                def finish_pair(state):
                    # row -> column layout for the output projection: one
                    # xbar transpose per (pair, output), SP ring.
                    _, _, _, or1, or2, hp = state
                    nc.sync.dma_start(
                        out=O1c[hp][:].rearrange("p (b c) -> p b c", c=P),
                        in_=or1[:].rearrange("p a b -> p (a b)"),
                        transpose=True,
                    )
                    nc.sync.dma_start(
                        out=O2c[hp][:].rearrange("p (b c) -> p b c", c=P),
                        in_=or2[:].rearrange("p a b -> p (a b)"),
                        transpose=True,
                    )

                prev = None
                for hp in range(NP + 1):
                    state = None
                    if hp < NP:
                        pair = (2 * hp, 2 * hp + 1)
                        e_sb = {h: [] for h in pair}
                        et_qc = {h: [] for h in pair}
                        # normalized row-layout outputs for this pair:
                        # [128 tokens-within-chunk, chunk, 128 j]
                        or1 = wp.tile([P, NQ, P], BF16, tag="or1", name="or1",
                                      bufs=2)
                        or2 = wp.tile([P, NQ, P], BF16, tag="or2", name="or2",
                                      bufs=2)
                        state = (pair, e_sb, et_qc, or1, or2, hp)
                        for qc in range(NQ):
                            # the two heads of a pair occupy array row groups
                            # 0-63 / 64-127 (lhsT partition base auto-derives
                            # tile_position); ALTERNATING their matmuls lets
                            # the two K=64 streams overlap on the PE
                            # (measured 211 vs 501 ns/matmul at N=512).
                            pse = {h: psA.tile([P, S], F32, tag="pe", name="pe")
                                   for h in pair}
                            for nb in range(NKB):
                                for h in pair:
                                    r0 = HD * (h % 2)
                                    nc.tensor.matmul(
                                        pse[h][:, nb * 512:(nb + 1) * 512],
                                        lhsT=K1_sb[hp][r0:r0 + HD,
                                                       qc * P:(qc + 1) * P],
                                        rhs=K2_sb[hp][r0:r0 + HD,
                                                      nb * 512:(nb + 1) * 512],
                                        start=True, stop=True,
                                    )
                            for h in pair:
                                e = wp.tile([P, S], BF16, tag=f"e{qc}",
                                            name=f"e{qc}", bufs=4)
                                nc.scalar.activation(e[:], pse[h][:], AF.Exp,
                                                     scale=0.125)
                                e_sb[h].append(e)
                                # E^T for this q-chunk via xbar DMA transpose
                                # into a PER-QC tile (so the tile ring
                                # rotates at site granularity), all on the
                                # SP HWDGE ring (the ACT ring corrupts
                                # transposed data on this runtime).
                                ett = wp.tile([P, NQ, P], BF16,
                                              tag=f"et{h % 2}_{qc}", name="ett",
                                              bufs=1)
                                nc.sync.dma_start(out=ett[:], in_=e[:],
                                                  transpose=True)
                                et_qc[h].append(ett)
                            # fill the PE idle window under this step's
                            # exps: previous pair's out2row site, then THIS
                            # pair's out1row for the previous step (its E^T
                            # transpose was issued one step ago and only
                            # depends on that single e tile).
                            if prev is not None:
                                emit_site_out2(prev, qc)
                            if qc >= 1:
                                emit_site_out1(state, qc - 1)
                        emit_site_out1(state, NQ - 1)
                    else:
                        # trailing phase: drain the last pair's out2row
                        for s in range(NQ):
                            emit_site_out2(prev, s)
                    if prev is not None:
                        finish_pair(prev)
                    prev = state

                # ---- output projections (head-pair K=128 contraction) -----------
                def out_proj(o_sb, wo_sb, bo_sb, y):
                    for mc in range(NCC):
                        ps = psA.tile([P, S], F32, tag="pe", name="pe")
                        for nb in range(NKB):
                            for hp in range(NP):
                                nc.tensor.matmul(
                                    ps[:, nb * 512:(nb + 1) * 512],
                                    lhsT=wo_sb[hp][:, mc * P:(mc + 1) * P],
                                    rhs=o_sb[hp][:, nb * 512:(nb + 1) * 512],
                                    start=(hp == 0), stop=(hp == NP - 1),
                                )
                        ysb = wp.tile([P, S], F32, tag="y", name="y")
                        nc.vector.tensor_scalar(ysb[:], ps[:],
                                                bo_sb[:, mc:mc + 1], None, ALU.add)
                        nc.gpsimd.dma_start(out=y[mc * P:(mc + 1) * P, :], in_=ysb[:])

                out_proj(O1c, wo1_sb, bo1_sb, y1)
                out_proj(O2c, wo2_sb, bo2_sb, y2)

_NC_CACHE: bacc.Bacc | None = None


def _compile(nc: bacc.Bacc) -> None:
    """nc.compile() with the ACT-table pass pinned to one table set.

    All activation funcs used here (Exp, Identity, Copy) live in the
    'natural_log_exp_and_others' set. The default insert_act_table_loads pass
    picks the first set containing each func, which can alternate sets and
    insert a LoadActFuncSet before nearly every activation (each very
    expensive on hardware). Restricting every other set to empty (keeping
    dict order, so set ids stay valid) makes every func resolve to the one
    set -> a single load.
    """
    import concourse.bacc as _bacc_mod

    orig = _bacc_mod.get_activation_tables
    keep = "natural_log_exp_and_others"

    def pinned(arch):
        tables = orig(arch)
        assert keep in tables
        return {k: (v if k == keep else set()) for k, v in tables.items()}

    _bacc_mod.get_activation_tables = pinned
    try:
        nc.compile()
    finally:
        _bacc_mod.get_activation_tables = orig


def build_nc() -> bacc.Bacc:
    global _NC_CACHE
    if _NC_CACHE is None:
        nc = bacc.Bacc("TRN2", target_bir_lowering=False, debug=False)
        _emit(nc)
        _compile(nc)
        _NC_CACHE = nc
    return _NC_CACHE


def make_in_maps(inputs: dict[str, np.ndarray]) -> list[dict[str, np.ndarray]]:
    bf = ml_dtypes.bfloat16
    i1 = np.asarray(inputs["input1"], np.float32).reshape(B, C, S)
    i2 = np.asarray(inputs["input2"], np.float32).reshape(B, C, S)
    k1_w = np.asarray(inputs["k1_w"], np.float32)
    k2_w = np.asarray(inputs["k2_w"], np.float32)
    v1_w = np.asarray(inputs["v1_w"], np.float32)
    v2_w = np.asarray(inputs["v2_w"], np.float32)
    o1_w = np.asarray(inputs["o1_w"], np.float32)
    o2_w = np.asarray(inputs["o2_w"], np.float32)
    k1_b = np.asarray(inputs["k1_b"], np.float32)
    k2_b = np.asarray(inputs["k2_b"], np.float32)
    v1_b = np.asarray(inputs["v1_b"], np.float32)
    v2_b = np.asarray(inputs["v2_b"], np.float32)
    o1_b = np.asarray(inputs["o1_b"], np.float32)
    o2_b = np.asarray(inputs["o2_b"], np.float32)

    shared = {
        "wk1": np.ascontiguousarray(k1_w.T).astype(bf),
        "wk2": np.ascontiguousarray(k2_w.T).astype(bf),
        "wv1": np.ascontiguousarray(v1_w.T).astype(bf),
        "wv2": np.ascontiguousarray(v2_w.T).astype(bf),
        "wo1": np.ascontiguousarray(o1_w.T).astype(bf),
        "wo2": np.ascontiguousarray(o2_w.T).astype(bf),
        "bk1": np.ascontiguousarray(k1_b.reshape(J // P, P).T),
        "bk2": np.ascontiguousarray(k2_b.reshape(J // P, P).T),
        # V-bias folds into the output-projection bias:
        #   out1 uses v2  ->  bo1_eff = o1_b + o1_w @ v2_b
        "bo1": np.ascontiguousarray((o1_b + o1_w @ v2_b).reshape(NCC, P).T),
        "bo2": np.ascontiguousarray((o2_b + o2_w @ v1_b).reshape(NCC, P).T),
    }
    return [
        {"x1": i1[b].astype(bf), "x2": i2[b].astype(bf), **shared}
        for b in range(B)
    ]


def kernel(**inputs) -> tuple[np.ndarray, np.ndarray]:
    from concourse.bass_utils import run_bass_kernel_spmd

    nc = build_nc()
    in_maps = make_in_maps(inputs)
    for _attempt in range(3):
        res = run_bass_kernel_spmd(nc, in_maps, list(range(B))).results
        out1 = np.stack([res[b]["y1"] for b in range(B)]).reshape(B, C, 32, 32)
        out2 = np.stack([res[b]["y2"] for b in range(B)]).reshape(B, C, 32, 32)
        # very rarely the first execution on a cold device returns NaNs;
        # re-running the same NEFF has always produced clean output
        if not (np.isnan(out1).any() or np.isnan(out2).any()):
            break
    return out1.astype(np.float32), out2.astype(np.float32)
